# revision 11
# baseline (speedup 1.0000x reference)
"""Trainium2 Bass kernel for AdaptiveGraphConv (per-(b,t) graph attention + BatchNorm2d).

Reference math (B=8, C=256, T=64, V=468, INTER=128, OUT=256):
    theta = einsum('bctv,ic->btvi', x, W_theta) + b_theta
    phi   = einsum('bctv,ic->btvi', x, W_phi)   + b_phi
    g     = einsum('bctv,oc->btvo', x, W_g)     + b_g
    A     = softmax(theta @ phi^T / sqrt(INTER), axis=-1)   # per (b,t), V x V
    out   = (A @ g) transposed to (B, OUT, T, V)
    out   = batchnorm2d(out, training stats over (B,T,V) per channel)

Sharding: data-parallel over B (1 batch per NeuronCore, 8 cores);
BN batch statistics are all-reduced across cores.

Device program (SPMD, per core, matmuls bf16 with fp32 PSUM accumulation):
  phase 1 (per t slice):
    thetaT[i,v], phiT[i,v] = W^T-chunks x x-chunks           (PE)
    S^T[w,v] = phiT-cols^T @ thetaT ; E^T = exp(S^T/sqrt(d)) (PE + ACT)
    g[w,o] = x-cols^T @ WgT                                  (PE + ACT/DVE cast)
    esum = sum of E^T chunks                                 (DVE)
    Z = partition_all_reduce(esum)  (broadcast to 128 parts) (GpSimd)
    zinv = reciprocal_approx(Z)                              (DVE)
    U^T[o,v] = g-cols^T @ E^T                                (PE)
    out = U^T * zinv  (+ channel sum via fused reduce)       (DVE TTR)
    sumsq via fused square-reduce                            (DVE TTR)
    out -> DRAM scratch (bf16)
  all-reduce (sum, sumsq) per channel -> BN affine (s, t)
  phase 2: stream scratch, out*s+t per channel, write output (DVE + DMA)
"""

import math

import numpy as np
import ml_dtypes

import concourse.bacc as bacc
import concourse.tile as tile
from concourse import mybir
from concourse.bass_utils import run_bass_kernel_spmd

B, C, T, V = 8, 256, 64, 468
INTER, OUT = 128, 256
BN_EPS = 1e-5
NCORES = 8
P = 128

SCALE = 1.0 / math.sqrt(INTER)
# w-axis chunks of V for 128-partition tiles
WCH = [(0, 128), (128, 128), (256, 128), (384, V - 384)]
T_BLK = 8  # t-slices per input DMA / phase-2 tile
N_TB = T // T_BLK
PREFETCH_GROUPS = 5  # (tb) groups x 2 oc tiles prefetched before the collective

F32 = mybir.dt.float32
BF16 = mybir.dt.bfloat16

TRACE = False
LAST_EXEC_NS = None

_CACHE = {}


def _build(with_bias: bool):
    nc = bacc.Bacc("TRN2", target_bir_lowering=False, debug=False, num_devices=NCORES)

    x_ext = nc.dram_tensor("x", [C, T, V], BF16, kind="ExternalInput").ap()
    wt_ext = nc.dram_tensor("wt", [2, P, INTER], BF16, kind="ExternalInput").ap()
    wp_ext = nc.dram_tensor("wp", [2, P, INTER], BF16, kind="ExternalInput").ap()
    wg_ext = nc.dram_tensor("wg", [2, P, OUT], BF16, kind="ExternalInput").ap()
    gb_ext = nc.dram_tensor("gb", [P, 4], F32, kind="ExternalInput").ap()
    if with_bias:
        bt_ext = nc.dram_tensor("bt", [INTER, 1], F32, kind="ExternalInput").ap()
        bp_ext = nc.dram_tensor("bp", [INTER, 1], F32, kind="ExternalInput").ap()
        bg_ext = nc.dram_tensor("bg", [1, OUT], F32, kind="ExternalInput").ap()
    out_ext = nc.dram_tensor("out", [OUT, T, V], F32, kind="ExternalOutput").ap()

    cnt_glob = float(NCORES * T * V)

    with tile.TileContext(nc) as tc:
        with (
            tc.tile_pool(name="consts", bufs=1) as consts,
            tc.tile_pool(name="xin", bufs=2) as xin,
            tc.tile_pool(name="thp", bufs=4) as thp,
            tc.tile_pool(name="ep", bufs=8) as ep,
            tc.tile_pool(name="gp", bufs=8) as gp,
            tc.tile_pool(name="esp", bufs=6) as esp,
            tc.tile_pool(name="zp", bufs=2) as zp,
            tc.tile_pool(name="op", bufs=4) as op_,
            tc.tile_pool(name="small", bufs=1) as small,
            tc.tile_pool(name="p2", bufs=2) as p2,
            tc.tile_pool(name="pp_th", bufs=2, space="PSUM") as pp_th,
            tc.tile_pool(name="pp_sg", bufs=3, space="PSUM") as pp_sg,
            tc.tile_pool(name="pp_z", bufs=1, space="PSUM") as pp_z,
            tc.tile_pool(name="pp_u", bufs=2, space="PSUM") as pp_u,
            tc.tile_pool(name="dram", bufs=1, space="DRAM") as dram,
        ):
            # ---- constants ----
            wt_sb = [consts.tile([P, INTER], BF16, tag=f"wt{k}", name=f"wt_sb{k}") for k in range(2)]
            wp_sb = [consts.tile([P, INTER], BF16, tag=f"wp{k}", name=f"wp_sb{k}") for k in range(2)]
            wg_sb = [consts.tile([P, OUT], BF16, tag=f"wg{k}", name=f"wg_sb{k}") for k in range(2)]
            for k in range(2):
                nc.sync.dma_start(out=wt_sb[k][:], in_=wt_ext[k])
                nc.sync.dma_start(out=wp_sb[k][:], in_=wp_ext[k])
                nc.sync.dma_start(out=wg_sb[k][:], in_=wg_ext[k])
            ones = consts.tile([P, P], BF16, tag="ones")
            nc.vector.memset(ones[:], 1.0)
            gb_sb = consts.tile([P, 4], F32, tag="gb")
            nc.sync.dma_start(out=gb_sb[:], in_=gb_ext[:])
            eps_sb = consts.tile([P, 1], F32, tag="eps")
            nc.vector.memset(eps_sb[:], BN_EPS)
            if with_bias:
                bt_sb = consts.tile([INTER, 1], F32, tag="bt")
                bp_sb = consts.tile([INTER, 1], F32, tag="bp")
                bg_sb = consts.tile([P, OUT], F32, tag="bg")
                nc.sync.dma_start(out=bt_sb[:], in_=bt_ext[:])
                nc.sync.dma_start(out=bp_sb[:], in_=bp_ext[:])
                nc.sync.dma_start(out=bg_sb[:], in_=bg_ext.to_broadcast([P, OUT]))

            # per-channel running stats (bn_stats 6-tuples per t-slice and o-chunk)
            stats_acc_t = consts.tile([P, T, 2, 6], F32, tag="stats", name="stats_acc_t")

            scr = [
                [dram.tile([P, T_BLK, V], BF16, name=f"scr{oc}_{tb}") for tb in range(N_TB)]
                for oc in range(2)
            ]

            # ---- phase 1 ----
            for tb in range(N_TB):
                xg = [xin.tile([P, T_BLK, V], BF16, tag=f"xg{k}", name=f"xg{k}") for k in range(2)]
                for k in range(2):
                    nc.sync.dma_start(
                        out=xg[k][:],
                        in_=x_ext[k * P : (k + 1) * P, tb * T_BLK : (tb + 1) * T_BLK, :],
                    )
                for tt in range(T_BLK):
                    t = tb * T_BLK + tt
                    xt = [xg[k][:, tt, :] for k in range(2)]

                    # thetaT / phiT : [INTER, V]
                    th_ps = pp_th.tile([P, V], F32, tag="th")
                    ph_ps = pp_th.tile([P, V], F32, tag="th")
                    for k in range(2):
                        nc.tensor.matmul(
                            th_ps[:], lhsT=wt_sb[k][:], rhs=xt[k],
                            start=(k == 0), stop=(k == 1),
                        )
                    for k in range(2):
                        nc.tensor.matmul(
                            ph_ps[:], lhsT=wp_sb[k][:], rhs=xt[k],
                            start=(k == 0), stop=(k == 1),
                        )
                    th_sb = thp.tile([P, V], BF16, tag="th_sb")
                    ph_sb = thp.tile([P, V], BF16, tag="th_sb")
                    if with_bias:
                        nc.scalar.activation(
                            th_sb[:], th_ps[:], mybir.ActivationFunctionType.Identity,
                            bias=bt_sb[:, 0:1],
                        )
                        nc.scalar.activation(
                            ph_sb[:], ph_ps[:], mybir.ActivationFunctionType.Identity,
                            bias=bp_sb[:, 0:1],
                        )
                    else:
                        nc.scalar.copy(th_sb[:], th_ps[:])
                        nc.scalar.copy(ph_sb[:], ph_ps[:])

                    # scores^T chunks + exp
                    e_sb = []
                    for (w0, wsz) in WCH:
                        s_ps = pp_sg.tile([P, V], F32, tag="sg")
                        nc.tensor.matmul(
                            s_ps[:wsz], lhsT=ph_sb[:, w0 : w0 + wsz], rhs=th_sb[:],
                            start=True, stop=True,
                        )
                        e_t = ep.tile([P, V], BF16, tag="e")
                        nc.scalar.activation(
                            e_t[:wsz], s_ps[:wsz],
                            mybir.ActivationFunctionType.Exp, scale=SCALE,
                        )
                        e_sb.append(e_t)

                    # g chunks: [w, OUT]
                    g_sb = []
                    for (w0, wsz) in WCH:
                        g_ps = pp_sg.tile([P, OUT], F32, tag="sg")
                        for k in range(2):
                            nc.tensor.matmul(
                                g_ps[:wsz], lhsT=xt[k][:, w0 : w0 + wsz], rhs=wg_sb[k][:],
                                start=(k == 0), stop=(k == 1),
                            )
                        g_t = gp.tile([P, OUT], BF16, tag="g")
                        if with_bias:
                            nc.vector.tensor_tensor(
                                g_t[:wsz], g_ps[:wsz], bg_sb[:wsz],
                                mybir.AluOpType.add,
                            )
                        else:
                            nc.scalar.copy(g_t[:wsz], g_ps[:wsz])
                        g_sb.append(g_t)

                    # softmax denominator: pre-fold E chunks pairwise on DVE,
                    # then 2 ones-matmuls reduce over partitions and broadcast
                    wsz3 = WCH[3][1]
                    e01 = esp.tile([P, V], BF16, tag="esum")
                    nc.vector.tensor_tensor(
                        e01[:], e_sb[0][:], e_sb[1][:], mybir.AluOpType.add
                    )
                    e23 = esp.tile([P, V], BF16, tag="esum")
                    nc.vector.tensor_copy(e23[64:], e_sb[2][64:])
                    nc.vector.tensor_tensor(
                        e23[:wsz3], e_sb[2][:wsz3], e_sb[3][:wsz3],
                        mybir.AluOpType.add,
                    )
                    esum = esp.tile([P, V], BF16, tag="esum")
                    nc.vector.tensor_tensor(
                        esum[:], e01[:], e23[:], mybir.AluOpType.add
                    )
                    z_ps = pp_z.tile([P, V], F32, tag="z")
                    nc.tensor.matmul(
                        z_ps[:], lhsT=ones[:], rhs=esum[:], start=True, stop=True
                    )
                    zinv = zp.tile([P, V], F32, tag="zinv")
                    nc.vector.reciprocal_approx_fast(out=zinv[:], in_=z_ps[:])

                    # U^T = A_unnorm @ g : [OUT(2x128), V]; normalize; stats; spill
                    o_sb = op_.tile([P, 2, V], BF16, tag="o")
                    for oc in range(2):
                        u_ps = pp_u.tile([P, V], F32, tag="u")
                        for wc, (w0, wsz) in enumerate(WCH):
                            nc.tensor.matmul(
                                u_ps[:],
                                lhsT=g_sb[wc][:wsz, oc * P : (oc + 1) * P],
                                rhs=e_sb[wc][:wsz, :],
                                start=(wc == 0), stop=(wc == 3),
                            )
                        nc.vector.tensor_tensor(
                            o_sb[:, oc, :], u_ps[:], zinv[:], mybir.AluOpType.mult
                        )
                        nc.vector.bn_stats(
                            out=stats_acc_t[:, t, oc, :], in_=o_sb[:, oc, :]
                        )
                        nc.sync.dma_start(out=scr[oc][tb][:, tt, :], in_=o_sb[:, oc, :])

            # ---- prefetch the first phase-2 input groups (overlaps collective) ----
            tins = {}
            for tb in range(PREFETCH_GROUPS):
                for oc in range(2):
                    tin = p2.tile([P, T_BLK, V], BF16, tag="p2in", bufs=12,
                                  name=f"tin_pre{oc}_{tb}")
                    nc.sync.dma_start(out=tin[:], in_=scr[oc][tb][:])
                    tins[(oc, tb)] = tin

            # ---- BN stats reduce + all-reduce ----
            cnt_local = float(T * V)
            pay = small.tile([P, 4], F32, tag="pay")
            tmp1 = small.tile([P, 2], F32, tag="tmp1")
            tmp2 = small.tile([P, 2], F32, tag="tmp2")
            for oc in range(2):
                mv = small.tile([P, 2], F32, tag=f"mv{oc}", name=f"mv{oc}")
                nc.vector.bn_aggr(out=mv[:], in_=stats_acc_t[:, :, oc, :])
                # sum = cnt*mean ; sumsq = cnt*(var + mean^2)
                nc.vector.tensor_scalar_mul(pay[:, oc : oc + 1], mv[:, 0:1], cnt_local)
                nc.vector.tensor_tensor(
                    tmp1[:, oc : oc + 1], mv[:, 0:1], mv[:, 0:1], mybir.AluOpType.mult
                )
                nc.vector.tensor_tensor(
                    tmp2[:, oc : oc + 1], mv[:, 1:2], tmp1[:, oc : oc + 1],
                    mybir.AluOpType.add,
                )
                nc.vector.tensor_scalar_mul(
                    pay[:, 2 + oc : 3 + oc], tmp2[:, oc : oc + 1], cnt_local
                )

            pay_dram = dram.tile([P, 4], F32)
            red_dram = dram.tile([P, 4], F32)
            nc.gpsimd.dma_start(out=pay_dram[:], in_=pay[:])
            nc.gpsimd.collective_compute(
                "AllReduce",
                mybir.AluOpType.add,
                replica_groups=[list(range(NCORES))],
                ins=[pay_dram.opt()],
                outs=[red_dram.opt()],
            )
            red = small.tile([P, 4], F32, tag="red")
            nc.gpsimd.dma_start(out=red[:], in_=red_dram[:])

            # mean = sum/N ; var = sumsq/N - mean^2 ; s = gamma/sqrt(var+eps)
            # t = beta - mean*s
            mean_g = small.tile([P, 2], F32, tag="mean_g")
            ex2 = small.tile([P, 2], F32, tag="ex2")
            var_g = small.tile([P, 2], F32, tag="var_g")
            rstd = small.tile([P, 2], F32, tag="rstd")
            s_vec = small.tile([P, 2], F32, tag="s_vec")
            t_vec = small.tile([P, 2], F32, tag="t_vec")
            nc.vector.tensor_scalar_mul(mean_g[:], red[:, 0:2], 1.0 / cnt_glob)
            nc.vector.tensor_scalar_mul(ex2[:], red[:, 2:4], 1.0 / cnt_glob)
            nc.vector.tensor_tensor(
                var_g[:], mean_g[:], mean_g[:], mybir.AluOpType.mult
            )
            nc.vector.tensor_tensor(
                var_g[:], ex2[:], var_g[:], mybir.AluOpType.subtract
            )
            nc.scalar.activation(
                rstd[:], var_g[:], mybir.ActivationFunctionType.Sqrt,
                bias=eps_sb[:, 0:1],
            )
            nc.vector.reciprocal(out=rstd[:], in_=rstd[:])
            nc.vector.tensor_tensor(s_vec[:], rstd[:], gb_sb[:, 0:2], mybir.AluOpType.mult)
            nc.vector.tensor_tensor(t_vec[:], mean_g[:], s_vec[:], mybir.AluOpType.mult)
            nc.vector.tensor_tensor(
                t_vec[:], gb_sb[:, 2:4], t_vec[:], mybir.AluOpType.subtract
            )

            # ---- phase 2: scratch -> affine -> out ----
            for tb in range(N_TB):
                for oc in range(2):
                    t0 = tb * T_BLK
                    if (oc, tb) in tins:
                        tin = tins.pop((oc, tb))
                    else:
                        tin = p2.tile([P, T_BLK, V], BF16, tag="p2in",
                                      bufs=12,
                                      name=f"tin{oc}_{tb}")
                        nc.sync.dma_start(out=tin[:], in_=scr[oc][tb][:])
                    tout = p2.tile([P, T_BLK, V], F32, tag="p2out", bufs=2)
                    nc.vector.tensor_scalar(
                        tout[:], tin[:],
                        s_vec[:, oc : oc + 1], t_vec[:, oc : oc + 1],
                        mybir.AluOpType.mult, mybir.AluOpType.add,
                    )
                    nc.sync.dma_start(
                        out=out_ext[oc * P : (oc + 1) * P, t0 : t0 + T_BLK, :],
                        in_=tout[:],
                    )

    nc.compile()
    return nc


def _get_nc(with_bias: bool):
    key = with_bias
    if key not in _CACHE:
        _CACHE[key] = _build(with_bias)
    return _CACHE[key]


def _ensure_ntff_hook():
    import sys, types
    import antenv

    if "antenv.axon_hooks" not in sys.modules:
        mod = types.ModuleType("antenv.axon_hooks")
        _h = [None]
        mod.set_axon_ntff_profile_hook = lambda h: _h.__setitem__(0, h)
        mod.get_axon_ntff_profile_hook = lambda: _h[0]
        sys.modules["antenv.axon_hooks"] = mod
        antenv.axon_hooks = mod
    mod = sys.modules["antenv.axon_hooks"]
    if mod.get_axon_ntff_profile_hook() is None:
        try:
            from trn_agent_boot.trn_boot import _ntff_profile_via_ctypes

            mod.set_axon_ntff_profile_hook(
                _ntff_profile_via_ctypes("/opt/axon/libaxon_pjrt.so")
            )
        except Exception:
            pass


def kernel(x, W_theta, b_theta, W_phi, b_phi, W_g, b_g, bn_gamma, bn_beta):
    global LAST_EXEC_NS
    x = np.asarray(x, dtype=np.float32)
    with_bias = bool(
        np.any(np.asarray(b_theta)) or np.any(np.asarray(b_phi)) or np.any(np.asarray(b_g))
    )

    x_bf = x.astype(ml_dtypes.bfloat16)  # (B, C, T, V)
    wt = np.ascontiguousarray(
        np.asarray(W_theta, dtype=np.float32).T.astype(ml_dtypes.bfloat16).reshape(2, P, INTER)
    )
    wp = np.ascontiguousarray(
        np.asarray(W_phi, dtype=np.float32).T.astype(ml_dtypes.bfloat16).reshape(2, P, INTER)
    )
    wg = np.ascontiguousarray(
        np.asarray(W_g, dtype=np.float32).T.astype(ml_dtypes.bfloat16).reshape(2, P, OUT)
    )
    gamma = np.asarray(bn_gamma, dtype=np.float32).reshape(2, P).T  # [128, 2]
    beta = np.asarray(bn_beta, dtype=np.float32).reshape(2, P).T
    gb = np.ascontiguousarray(np.concatenate([gamma, beta], axis=1))  # [128, 4]

    nc = _get_nc(with_bias)

    in_maps = []
    for b in range(NCORES):
        m = {
            "x": np.ascontiguousarray(x_bf[b]),
            "wt": wt,
            "wp": wp,
            "wg": wg,
            "gb": gb,
        }
        if with_bias:
            m["bt"] = np.asarray(b_theta, dtype=np.float32).reshape(INTER, 1)
            m["bp"] = np.asarray(b_phi, dtype=np.float32).reshape(INTER, 1)
            m["bg"] = np.asarray(b_g, dtype=np.float32).reshape(1, OUT)
        in_maps.append(m)

    if TRACE:
        _ensure_ntff_hook()
    r = run_bass_kernel_spmd(nc, in_maps, list(range(NCORES)), trace=TRACE)
    LAST_EXEC_NS = r.exec_time_ns

    out = np.stack([r.results[b]["out"] for b in range(NCORES)], axis=0)
    return out.astype(np.float32)


# revision 12
# speedup vs baseline: 1.0183x; 1.0183x over previous
"""Trainium2 Bass kernel for AdaptiveGraphConv (per-(b,t) graph attention + BatchNorm2d).

Reference math (B=8, C=256, T=64, V=468, INTER=128, OUT=256):
    theta = einsum('bctv,ic->btvi', x, W_theta) + b_theta
    phi   = einsum('bctv,ic->btvi', x, W_phi)   + b_phi
    g     = einsum('bctv,oc->btvo', x, W_g)     + b_g
    A     = softmax(theta @ phi^T / sqrt(INTER), axis=-1)   # per (b,t), V x V
    out   = (A @ g) transposed to (B, OUT, T, V)
    out   = batchnorm2d(out, training stats over (B,T,V) per channel)

Sharding: data-parallel over B (1 batch per NeuronCore, 8 cores);
BN batch statistics are all-reduced across cores.

Device program (SPMD, per core, matmuls bf16 with fp32 PSUM accumulation):
  phase 1 (per t slice):
    thetaT[i,v], phiT[i,v] = W^T-chunks x x-chunks           (PE)
    S^T[w,v] = phiT-cols^T @ thetaT ; E^T = exp(S^T/sqrt(d)) (PE + ACT)
    g[w,o] = x-cols^T @ WgT                                  (PE + ACT/DVE cast)
    esum = sum of E^T chunks                                 (DVE)
    Z = partition_all_reduce(esum)  (broadcast to 128 parts) (GpSimd)
    zinv = reciprocal_approx(Z)                              (DVE)
    U^T[o,v] = g-cols^T @ E^T                                (PE)
    out = U^T * zinv  (+ channel sum via fused reduce)       (DVE TTR)
    sumsq via fused square-reduce                            (DVE TTR)
    out -> DRAM scratch (bf16)
  all-reduce (sum, sumsq) per channel -> BN affine (s, t)
  phase 2: stream scratch, out*s+t per channel, write output (DVE + DMA)
"""

import math

import numpy as np
import ml_dtypes

import concourse.bacc as bacc
import concourse.tile as tile
from concourse import mybir
from concourse.bass_utils import run_bass_kernel_spmd

B, C, T, V = 8, 256, 64, 468
INTER, OUT = 128, 256
BN_EPS = 1e-5
NCORES = 8
P = 128

SCALE = 1.0 / math.sqrt(INTER)
# w-axis chunks of V for 128-partition tiles
WCH = [(0, 128), (128, 128), (256, 128), (384, V - 384)]
T_BLK = 8  # t-slices per input DMA / phase-2 tile
N_TB = T // T_BLK
PREFETCH_GROUPS = 5  # (tb) groups x 2 oc tiles prefetched before the collective

F32 = mybir.dt.float32
BF16 = mybir.dt.bfloat16

TRACE = False
LAST_EXEC_NS = None

_CACHE = {}


def _build(with_bias: bool):
    nc = bacc.Bacc("TRN2", target_bir_lowering=False, debug=False, num_devices=NCORES)

    x_ext = nc.dram_tensor("x", [C, T, V], BF16, kind="ExternalInput").ap()
    wt_ext = nc.dram_tensor("wt", [2, P, INTER], BF16, kind="ExternalInput").ap()
    wp_ext = nc.dram_tensor("wp", [2, P, INTER], BF16, kind="ExternalInput").ap()
    wg_ext = nc.dram_tensor("wg", [2, P, OUT], BF16, kind="ExternalInput").ap()
    gb_ext = nc.dram_tensor("gb", [P, 4], F32, kind="ExternalInput").ap()
    if with_bias:
        bt_ext = nc.dram_tensor("bt", [INTER, 1], F32, kind="ExternalInput").ap()
        bp_ext = nc.dram_tensor("bp", [INTER, 1], F32, kind="ExternalInput").ap()
        bg_ext = nc.dram_tensor("bg", [1, OUT], F32, kind="ExternalInput").ap()
    out_ext = nc.dram_tensor("out", [OUT, T, V], F32, kind="ExternalOutput").ap()

    cnt_glob = float(NCORES * T * V)

    with tile.TileContext(nc) as tc:
        with (
            tc.tile_pool(name="consts", bufs=1) as consts,
            tc.tile_pool(name="xin", bufs=2) as xin,
            tc.tile_pool(name="thp", bufs=4) as thp,
            tc.tile_pool(name="ep", bufs=8) as ep,
            tc.tile_pool(name="gp", bufs=8) as gp,
            tc.tile_pool(name="esp", bufs=6) as esp,
            tc.tile_pool(name="zp", bufs=2) as zp,
            tc.tile_pool(name="op", bufs=4) as op_,
            tc.tile_pool(name="small", bufs=1) as small,
            tc.tile_pool(name="p2", bufs=2) as p2,
            tc.tile_pool(name="pp_th", bufs=2, space="PSUM") as pp_th,
            tc.tile_pool(name="pp_sg", bufs=3, space="PSUM") as pp_sg,
            tc.tile_pool(name="pp_z", bufs=1, space="PSUM") as pp_z,
            tc.tile_pool(name="pp_u", bufs=2, space="PSUM") as pp_u,
            tc.tile_pool(name="dram", bufs=1, space="DRAM") as dram,
        ):
            # ---- constants ----
            wt_sb = [consts.tile([P, INTER], BF16, tag=f"wt{k}", name=f"wt_sb{k}") for k in range(2)]
            wp_sb = [consts.tile([P, INTER], BF16, tag=f"wp{k}", name=f"wp_sb{k}") for k in range(2)]
            wg_sb = [consts.tile([P, OUT], BF16, tag=f"wg{k}", name=f"wg_sb{k}") for k in range(2)]
            for k in range(2):
                nc.sync.dma_start(out=wt_sb[k][:], in_=wt_ext[k])
                nc.sync.dma_start(out=wp_sb[k][:], in_=wp_ext[k])
                nc.sync.dma_start(out=wg_sb[k][:], in_=wg_ext[k])
            ones = consts.tile([P, P], BF16, tag="ones")
            nc.vector.memset(ones[:], 1.0)
            gb_sb = consts.tile([P, 4], F32, tag="gb")
            nc.sync.dma_start(out=gb_sb[:], in_=gb_ext[:])
            eps_sb = consts.tile([P, 1], F32, tag="eps")
            nc.vector.memset(eps_sb[:], BN_EPS)
            if with_bias:
                bt_sb = consts.tile([INTER, 1], F32, tag="bt")
                bp_sb = consts.tile([INTER, 1], F32, tag="bp")
                bg_sb = consts.tile([P, OUT], F32, tag="bg")
                nc.sync.dma_start(out=bt_sb[:], in_=bt_ext[:])
                nc.sync.dma_start(out=bp_sb[:], in_=bp_ext[:])
                nc.sync.dma_start(out=bg_sb[:], in_=bg_ext.to_broadcast([P, OUT]))

            # per-channel running stats (bn_stats 6-tuples per t-slice and o-chunk)
            stats_acc_t = consts.tile([P, T, 2, 6], F32, tag="stats", name="stats_acc_t")

            scr = [
                [dram.tile([P, T_BLK, V], BF16, name=f"scr{oc}_{tb}") for tb in range(N_TB)]
                for oc in range(2)
            ]

            # ---- phase 1 ----
            for tb in range(N_TB):
                xg = [xin.tile([P, T_BLK, V], BF16, tag=f"xg{k}", name=f"xg{k}") for k in range(2)]
                for k in range(2):
                    nc.sync.dma_start(
                        out=xg[k][:],
                        in_=x_ext[k * P : (k + 1) * P, tb * T_BLK : (tb + 1) * T_BLK, :],
                    )
                for tt in range(T_BLK):
                    t = tb * T_BLK + tt
                    xt = [xg[k][:, tt, :] for k in range(2)]

                    # thetaT / phiT : [INTER, V]
                    th_ps = pp_th.tile([P, V], F32, tag="th")
                    ph_ps = pp_th.tile([P, V], F32, tag="th")
                    for k in range(2):
                        nc.tensor.matmul(
                            th_ps[:], lhsT=wt_sb[k][:], rhs=xt[k],
                            start=(k == 0), stop=(k == 1),
                        )
                    for k in range(2):
                        nc.tensor.matmul(
                            ph_ps[:], lhsT=wp_sb[k][:], rhs=xt[k],
                            start=(k == 0), stop=(k == 1),
                        )
                    th_sb = thp.tile([P, V], BF16, tag="th_sb")
                    ph_sb = thp.tile([P, V], BF16, tag="th_sb")
                    if with_bias:
                        nc.scalar.activation(
                            th_sb[:], th_ps[:], mybir.ActivationFunctionType.Identity,
                            bias=bt_sb[:, 0:1],
                        )
                        nc.scalar.activation(
                            ph_sb[:], ph_ps[:], mybir.ActivationFunctionType.Identity,
                            bias=bp_sb[:, 0:1],
                        )
                    else:
                        nc.scalar.copy(th_sb[:], th_ps[:])
                        nc.scalar.copy(ph_sb[:], ph_ps[:])

                    # scores^T chunks + exp
                    e_sb = []
                    for (w0, wsz) in WCH:
                        s_ps = pp_sg.tile([P, V], F32, tag="sg")
                        nc.tensor.matmul(
                            s_ps[:wsz], lhsT=ph_sb[:, w0 : w0 + wsz], rhs=th_sb[:],
                            start=True, stop=True,
                        )
                        e_t = ep.tile([P, V], BF16, tag="e")
                        nc.scalar.activation(
                            e_t[:wsz], s_ps[:wsz],
                            mybir.ActivationFunctionType.Exp, scale=SCALE,
                        )
                        e_sb.append(e_t)

                    # g chunks: [w, OUT]
                    g_sb = []
                    for (w0, wsz) in WCH:
                        g_ps = pp_sg.tile([P, OUT], F32, tag="sg")
                        for k in range(2):
                            nc.tensor.matmul(
                                g_ps[:wsz], lhsT=xt[k][:, w0 : w0 + wsz], rhs=wg_sb[k][:],
                                start=(k == 0), stop=(k == 1),
                            )
                        g_t = gp.tile([P, OUT], BF16, tag="g")
                        if with_bias:
                            nc.vector.tensor_tensor(
                                g_t[:wsz], g_ps[:wsz], bg_sb[:wsz],
                                mybir.AluOpType.add,
                            )
                        else:
                            nc.scalar.copy(g_t[:wsz], g_ps[:wsz])
                        g_sb.append(g_t)

                    # softmax denominator: pre-fold E chunks pairwise on DVE,
                    # then 2 ones-matmuls reduce over partitions and broadcast
                    wsz3 = WCH[3][1]
                    e01 = esp.tile([P, V], BF16, tag="esum")
                    nc.vector.tensor_tensor(
                        e01[:], e_sb[0][:], e_sb[1][:], mybir.AluOpType.add
                    )
                    e23 = esp.tile([P, V], BF16, tag="esum")
                    nc.vector.tensor_copy(e23[64:], e_sb[2][64:])
                    nc.vector.tensor_tensor(
                        e23[:wsz3], e_sb[2][:wsz3], e_sb[3][:wsz3],
                        mybir.AluOpType.add,
                    )
                    esum = esp.tile([P, V], BF16, tag="esum")
                    nc.vector.tensor_tensor(
                        esum[:], e01[:], e23[:], mybir.AluOpType.add
                    )
                    z_ps = pp_z.tile([P, V], F32, tag="z")
                    nc.tensor.matmul(
                        z_ps[:], lhsT=ones[:], rhs=esum[:], start=True, stop=True
                    )
                    zinv = zp.tile([P, V], F32, tag="zinv")
                    nc.vector.reciprocal_approx_fast(out=zinv[:], in_=z_ps[:])

                    # U^T = A_unnorm @ g : [OUT(2x128), V]; normalize; stats; spill
                    o_sb = op_.tile([P, 2, V], BF16, tag="o")
                    for oc in range(2):
                        u_ps = pp_u.tile([P, V], F32, tag="u")
                        for wc, (w0, wsz) in enumerate(WCH):
                            nc.tensor.matmul(
                                u_ps[:],
                                lhsT=g_sb[wc][:wsz, oc * P : (oc + 1) * P],
                                rhs=e_sb[wc][:wsz, :],
                                start=(wc == 0), stop=(wc == 3),
                            )
                        nc.vector.tensor_tensor(
                            o_sb[:, oc, :], u_ps[:], zinv[:], mybir.AluOpType.mult
                        )
                        nc.vector.bn_stats(
                            out=stats_acc_t[:, t, oc, :], in_=o_sb[:, oc, :]
                        )
                        nc.sync.dma_start(out=scr[oc][tb][:, tt, :], in_=o_sb[:, oc, :])

            # ---- prefetch the first phase-2 input groups (overlaps collective) ----
            tins = {}
            for tb in range(PREFETCH_GROUPS):
                for oc in range(2):
                    tin = p2.tile([P, T_BLK, V], BF16, tag="p2in", bufs=11,
                                  name=f"tin_pre{oc}_{tb}")
                    nc.sync.dma_start(out=tin[:], in_=scr[oc][tb][:])
                    tins[(oc, tb)] = tin

            # ---- BN stats reduce + all-reduce ----
            cnt_local = float(T * V)
            pay = small.tile([P, 4], F32, tag="pay")
            tmp1 = small.tile([P, 2], F32, tag="tmp1")
            tmp2 = small.tile([P, 2], F32, tag="tmp2")
            for oc in range(2):
                mv = small.tile([P, 2], F32, tag=f"mv{oc}", name=f"mv{oc}")
                nc.vector.bn_aggr(out=mv[:], in_=stats_acc_t[:, :, oc, :])
                # sum = cnt*mean ; sumsq = cnt*(var + mean^2)
                nc.vector.tensor_scalar_mul(pay[:, oc : oc + 1], mv[:, 0:1], cnt_local)
                nc.vector.tensor_tensor(
                    tmp1[:, oc : oc + 1], mv[:, 0:1], mv[:, 0:1], mybir.AluOpType.mult
                )
                nc.vector.tensor_tensor(
                    tmp2[:, oc : oc + 1], mv[:, 1:2], tmp1[:, oc : oc + 1],
                    mybir.AluOpType.add,
                )
                nc.vector.tensor_scalar_mul(
                    pay[:, 2 + oc : 3 + oc], tmp2[:, oc : oc + 1], cnt_local
                )

            pay_dram = dram.tile([P, 4], F32)
            red_dram = dram.tile([P, 4], F32)
            nc.gpsimd.dma_start(out=pay_dram[:], in_=pay[:])
            nc.gpsimd.collective_compute(
                "AllReduce",
                mybir.AluOpType.add,
                replica_groups=[list(range(NCORES))],
                ins=[pay_dram.opt()],
                outs=[red_dram.opt()],
            )
            red = small.tile([P, 4], F32, tag="red")
            nc.gpsimd.dma_start(out=red[:], in_=red_dram[:])

            # mean = sum/N ; var = sumsq/N - mean^2 ; s = gamma/sqrt(var+eps)
            # t = beta - mean*s
            mean_g = small.tile([P, 2], F32, tag="mean_g")
            ex2 = small.tile([P, 2], F32, tag="ex2")
            var_g = small.tile([P, 2], F32, tag="var_g")
            rstd = small.tile([P, 2], F32, tag="rstd")
            s_vec = small.tile([P, 2], F32, tag="s_vec")
            t_vec = small.tile([P, 2], F32, tag="t_vec")
            nc.vector.tensor_scalar_mul(mean_g[:], red[:, 0:2], 1.0 / cnt_glob)
            nc.vector.tensor_scalar_mul(ex2[:], red[:, 2:4], 1.0 / cnt_glob)
            nc.vector.tensor_tensor(
                var_g[:], mean_g[:], mean_g[:], mybir.AluOpType.mult
            )
            nc.vector.tensor_tensor(
                var_g[:], ex2[:], var_g[:], mybir.AluOpType.subtract
            )
            nc.scalar.activation(
                rstd[:], var_g[:], mybir.ActivationFunctionType.Sqrt,
                bias=eps_sb[:, 0:1],
            )
            nc.vector.reciprocal(out=rstd[:], in_=rstd[:])
            nc.vector.tensor_tensor(s_vec[:], rstd[:], gb_sb[:, 0:2], mybir.AluOpType.mult)
            nc.vector.tensor_tensor(t_vec[:], mean_g[:], s_vec[:], mybir.AluOpType.mult)
            nc.vector.tensor_tensor(
                t_vec[:], gb_sb[:, 2:4], t_vec[:], mybir.AluOpType.subtract
            )

            # ---- phase 2: scratch -> affine -> out ----
            for tb in range(N_TB):
                for oc in range(2):
                    t0 = tb * T_BLK
                    if (oc, tb) in tins:
                        tin = tins.pop((oc, tb))
                    else:
                        tin = p2.tile([P, T_BLK, V], BF16, tag="p2in",
                                      bufs=11,
                                      name=f"tin{oc}_{tb}")
                        nc.sync.dma_start(out=tin[:], in_=scr[oc][tb][:])
                    tout = p2.tile([P, T_BLK, V], F32, tag="p2out", bufs=3)
                    nc.vector.tensor_scalar(
                        tout[:], tin[:],
                        s_vec[:, oc : oc + 1], t_vec[:, oc : oc + 1],
                        mybir.AluOpType.mult, mybir.AluOpType.add,
                    )
                    nc.sync.dma_start(
                        out=out_ext[oc * P : (oc + 1) * P, t0 : t0 + T_BLK, :],
                        in_=tout[:],
                    )

    nc.compile()
    return nc


def _get_nc(with_bias: bool):
    key = with_bias
    if key not in _CACHE:
        _CACHE[key] = _build(with_bias)
    return _CACHE[key]


def _ensure_ntff_hook():
    import sys, types
    import antenv

    if "antenv.axon_hooks" not in sys.modules:
        mod = types.ModuleType("antenv.axon_hooks")
        _h = [None]
        mod.set_axon_ntff_profile_hook = lambda h: _h.__setitem__(0, h)
        mod.get_axon_ntff_profile_hook = lambda: _h[0]
        sys.modules["antenv.axon_hooks"] = mod
        antenv.axon_hooks = mod
    mod = sys.modules["antenv.axon_hooks"]
    if mod.get_axon_ntff_profile_hook() is None:
        try:
            from trn_agent_boot.trn_boot import _ntff_profile_via_ctypes

            mod.set_axon_ntff_profile_hook(
                _ntff_profile_via_ctypes("/opt/axon/libaxon_pjrt.so")
            )
        except Exception:
            pass


def kernel(x, W_theta, b_theta, W_phi, b_phi, W_g, b_g, bn_gamma, bn_beta):
    global LAST_EXEC_NS
    x = np.asarray(x, dtype=np.float32)
    with_bias = bool(
        np.any(np.asarray(b_theta)) or np.any(np.asarray(b_phi)) or np.any(np.asarray(b_g))
    )

    x_bf = x.astype(ml_dtypes.bfloat16)  # (B, C, T, V)
    wt = np.ascontiguousarray(
        np.asarray(W_theta, dtype=np.float32).T.astype(ml_dtypes.bfloat16).reshape(2, P, INTER)
    )
    wp = np.ascontiguousarray(
        np.asarray(W_phi, dtype=np.float32).T.astype(ml_dtypes.bfloat16).reshape(2, P, INTER)
    )
    wg = np.ascontiguousarray(
        np.asarray(W_g, dtype=np.float32).T.astype(ml_dtypes.bfloat16).reshape(2, P, OUT)
    )
    gamma = np.asarray(bn_gamma, dtype=np.float32).reshape(2, P).T  # [128, 2]
    beta = np.asarray(bn_beta, dtype=np.float32).reshape(2, P).T
    gb = np.ascontiguousarray(np.concatenate([gamma, beta], axis=1))  # [128, 4]

    nc = _get_nc(with_bias)

    in_maps = []
    for b in range(NCORES):
        m = {
            "x": np.ascontiguousarray(x_bf[b]),
            "wt": wt,
            "wp": wp,
            "wg": wg,
            "gb": gb,
        }
        if with_bias:
            m["bt"] = np.asarray(b_theta, dtype=np.float32).reshape(INTER, 1)
            m["bp"] = np.asarray(b_phi, dtype=np.float32).reshape(INTER, 1)
            m["bg"] = np.asarray(b_g, dtype=np.float32).reshape(1, OUT)
        in_maps.append(m)

    if TRACE:
        _ensure_ntff_hook()
    r = run_bass_kernel_spmd(nc, in_maps, list(range(NCORES)), trace=TRACE)
    LAST_EXEC_NS = r.exec_time_ns

    out = np.stack([r.results[b]["out"] for b in range(NCORES)], axis=0)
    return out.astype(np.float32)


# revision 14
# speedup vs baseline: 1.0490x; 1.0302x over previous
"""Trainium2 Bass kernel for AdaptiveGraphConv (per-(b,t) graph attention + BatchNorm2d).

Reference math (B=8, C=256, T=64, V=468, INTER=128, OUT=256):
    theta = einsum('bctv,ic->btvi', x, W_theta) + b_theta
    phi   = einsum('bctv,ic->btvi', x, W_phi)   + b_phi
    g     = einsum('bctv,oc->btvo', x, W_g)     + b_g
    A     = softmax(theta @ phi^T / sqrt(INTER), axis=-1)   # per (b,t), V x V
    out   = (A @ g) transposed to (B, OUT, T, V)
    out   = batchnorm2d(out, training stats over (B,T,V) per channel)

Sharding: data-parallel over B (1 batch per NeuronCore, 8 cores);
BN batch statistics are all-reduced across cores.

Device program (SPMD, per core, matmuls bf16 with fp32 PSUM accumulation):
  phase 1 (per t slice):
    thetaT[i,v], phiT[i,v] = W^T-chunks x x-chunks           (PE)
    S^T[w,v] = phiT-cols^T @ thetaT ; E^T = exp(S^T/sqrt(d)) (PE + ACT)
    g[w,o] = x-cols^T @ WgT                                  (PE + ACT/DVE cast)
    esum = sum of E^T chunks                                 (DVE)
    Z = partition_all_reduce(esum)  (broadcast to 128 parts) (GpSimd)
    zinv = reciprocal_approx(Z)                              (DVE)
    U^T[o,v] = g-cols^T @ E^T                                (PE)
    out = U^T * zinv  (+ channel sum via fused reduce)       (DVE TTR)
    sumsq via fused square-reduce                            (DVE TTR)
    out -> DRAM scratch (bf16)
  all-reduce (sum, sumsq) per channel -> BN affine (s, t)
  phase 2: stream scratch, out*s+t per channel, write output (DVE + DMA)
"""

import math

import numpy as np
import ml_dtypes

import concourse.bacc as bacc
import concourse.tile as tile
from concourse import mybir
from concourse.bass_utils import run_bass_kernel_spmd

B, C, T, V = 8, 256, 64, 468
INTER, OUT = 128, 256
BN_EPS = 1e-5
NCORES = 8
P = 128

SCALE = 1.0 / math.sqrt(INTER)
# w-axis chunks of V for 128-partition tiles
WCH = [(0, 128), (128, 128), (256, 128), (384, V - 384)]
T_BLK = 8  # t-slices per input DMA / phase-2 tile
N_TB = T // T_BLK
PREFETCH_GROUPS = 3
RESIDENT_TBS = [5, 6, 7]  # t-groups whose outputs stay in SBUF (no scratch round-trip)  # (tb) groups x 2 oc tiles prefetched before the collective

F32 = mybir.dt.float32
BF16 = mybir.dt.bfloat16

TRACE = False
LAST_EXEC_NS = None

_CACHE = {}


def _build(with_bias: bool):
    nc = bacc.Bacc("TRN2", target_bir_lowering=False, debug=False, num_devices=NCORES)

    x_ext = nc.dram_tensor("x", [C, T, V], BF16, kind="ExternalInput").ap()
    wt_ext = nc.dram_tensor("wt", [2, P, INTER], BF16, kind="ExternalInput").ap()
    wp_ext = nc.dram_tensor("wp", [2, P, INTER], BF16, kind="ExternalInput").ap()
    wg_ext = nc.dram_tensor("wg", [2, P, OUT], BF16, kind="ExternalInput").ap()
    gb_ext = nc.dram_tensor("gb", [P, 4], F32, kind="ExternalInput").ap()
    if with_bias:
        bt_ext = nc.dram_tensor("bt", [INTER, 1], F32, kind="ExternalInput").ap()
        bp_ext = nc.dram_tensor("bp", [INTER, 1], F32, kind="ExternalInput").ap()
        bg_ext = nc.dram_tensor("bg", [1, OUT], F32, kind="ExternalInput").ap()
    out_ext = nc.dram_tensor("out", [OUT, T, V], F32, kind="ExternalOutput").ap()

    cnt_glob = float(NCORES * T * V)

    with tile.TileContext(nc) as tc:
        with (
            tc.tile_pool(name="consts", bufs=1) as consts,
            tc.tile_pool(name="xin", bufs=2) as xin,
            tc.tile_pool(name="thp", bufs=4) as thp,
            tc.tile_pool(name="ep", bufs=8) as ep,
            tc.tile_pool(name="gp", bufs=8) as gp,
            tc.tile_pool(name="esp", bufs=6) as esp,
            tc.tile_pool(name="zp", bufs=2) as zp,
            tc.tile_pool(name="op", bufs=4) as op_,
            tc.tile_pool(name="small", bufs=1) as small,
            tc.tile_pool(name="p2", bufs=2) as p2,
            tc.tile_pool(name="pp_th", bufs=2, space="PSUM") as pp_th,
            tc.tile_pool(name="pp_sg", bufs=3, space="PSUM") as pp_sg,
            tc.tile_pool(name="pp_z", bufs=1, space="PSUM") as pp_z,
            tc.tile_pool(name="pp_u", bufs=2, space="PSUM") as pp_u,
            tc.tile_pool(name="dram", bufs=1, space="DRAM") as dram,
        ):
            # ---- constants ----
            wt_sb = [consts.tile([P, INTER], BF16, tag=f"wt{k}", name=f"wt_sb{k}") for k in range(2)]
            wp_sb = [consts.tile([P, INTER], BF16, tag=f"wp{k}", name=f"wp_sb{k}") for k in range(2)]
            wg_sb = [consts.tile([P, OUT], BF16, tag=f"wg{k}", name=f"wg_sb{k}") for k in range(2)]
            for k in range(2):
                nc.sync.dma_start(out=wt_sb[k][:], in_=wt_ext[k])
                nc.sync.dma_start(out=wp_sb[k][:], in_=wp_ext[k])
                nc.sync.dma_start(out=wg_sb[k][:], in_=wg_ext[k])
            ones = consts.tile([P, P], BF16, tag="ones")
            nc.vector.memset(ones[:], 1.0)
            gb_sb = consts.tile([P, 4], F32, tag="gb")
            nc.sync.dma_start(out=gb_sb[:], in_=gb_ext[:])
            eps_sb = consts.tile([P, 1], F32, tag="eps")
            nc.vector.memset(eps_sb[:], BN_EPS)
            if with_bias:
                bt_sb = consts.tile([INTER, 1], F32, tag="bt")
                bp_sb = consts.tile([INTER, 1], F32, tag="bp")
                bg_sb = consts.tile([P, OUT], F32, tag="bg")
                nc.sync.dma_start(out=bt_sb[:], in_=bt_ext[:])
                nc.sync.dma_start(out=bp_sb[:], in_=bp_ext[:])
                nc.sync.dma_start(out=bg_sb[:], in_=bg_ext.to_broadcast([P, OUT]))

            # per-channel running stats (bn_stats 6-tuples per t-slice and o-chunk)
            stats_acc_t = consts.tile([P, T, 2, 6], F32, tag="stats", name="stats_acc_t")

            scr = [
                [dram.tile([P, T_BLK, V], BF16, name=f"scr{oc}_{tb}") for tb in range(N_TB)]
                for oc in range(2)
            ]
            obig = {
                (oc, tb): consts.tile([P, T_BLK, V], BF16, tag=f"obig{oc}_{tb}",
                                      name=f"obig{oc}_{tb}")
                for oc in range(2) for tb in RESIDENT_TBS
            }

            # ---- phase 1 ----
            for tb in range(N_TB):
                xg = [xin.tile([P, T_BLK, V], BF16, tag=f"xg{k}", name=f"xg{k}") for k in range(2)]
                for k in range(2):
                    nc.sync.dma_start(
                        out=xg[k][:],
                        in_=x_ext[k * P : (k + 1) * P, tb * T_BLK : (tb + 1) * T_BLK, :],
                    )
                for tt in range(T_BLK):
                    t = tb * T_BLK + tt
                    xt = [xg[k][:, tt, :] for k in range(2)]

                    # thetaT / phiT : [INTER, V]
                    th_ps = pp_th.tile([P, V], F32, tag="th")
                    ph_ps = pp_th.tile([P, V], F32, tag="th")
                    for k in range(2):
                        nc.tensor.matmul(
                            th_ps[:], lhsT=wt_sb[k][:], rhs=xt[k],
                            start=(k == 0), stop=(k == 1),
                        )
                    for k in range(2):
                        nc.tensor.matmul(
                            ph_ps[:], lhsT=wp_sb[k][:], rhs=xt[k],
                            start=(k == 0), stop=(k == 1),
                        )
                    th_sb = thp.tile([P, V], BF16, tag="th_sb")
                    ph_sb = thp.tile([P, V], BF16, tag="th_sb")
                    if with_bias:
                        nc.scalar.activation(
                            th_sb[:], th_ps[:], mybir.ActivationFunctionType.Identity,
                            bias=bt_sb[:, 0:1],
                        )
                        nc.scalar.activation(
                            ph_sb[:], ph_ps[:], mybir.ActivationFunctionType.Identity,
                            bias=bp_sb[:, 0:1],
                        )
                    else:
                        nc.scalar.copy(th_sb[:], th_ps[:])
                        nc.scalar.copy(ph_sb[:], ph_ps[:])

                    # scores^T chunks + exp
                    e_sb = []
                    for (w0, wsz) in WCH:
                        s_ps = pp_sg.tile([P, V], F32, tag="sg")
                        nc.tensor.matmul(
                            s_ps[:wsz], lhsT=ph_sb[:, w0 : w0 + wsz], rhs=th_sb[:],
                            start=True, stop=True,
                        )
                        e_t = ep.tile([P, V], BF16, tag="e")
                        nc.scalar.activation(
                            e_t[:wsz], s_ps[:wsz],
                            mybir.ActivationFunctionType.Exp, scale=SCALE,
                        )
                        e_sb.append(e_t)

                    # g chunks: [w, OUT]
                    g_sb = []
                    for (w0, wsz) in WCH:
                        g_ps = pp_sg.tile([P, OUT], F32, tag="sg")
                        for k in range(2):
                            nc.tensor.matmul(
                                g_ps[:wsz], lhsT=xt[k][:, w0 : w0 + wsz], rhs=wg_sb[k][:],
                                start=(k == 0), stop=(k == 1),
                            )
                        g_t = gp.tile([P, OUT], BF16, tag="g")
                        if with_bias:
                            nc.vector.tensor_tensor(
                                g_t[:wsz], g_ps[:wsz], bg_sb[:wsz],
                                mybir.AluOpType.add,
                            )
                        else:
                            nc.scalar.copy(g_t[:wsz], g_ps[:wsz])
                        g_sb.append(g_t)

                    # softmax denominator: pre-fold E chunks pairwise on DVE,
                    # then 2 ones-matmuls reduce over partitions and broadcast
                    wsz3 = WCH[3][1]
                    e01 = esp.tile([P, V], BF16, tag="esum")
                    nc.vector.tensor_tensor(
                        e01[:], e_sb[0][:], e_sb[1][:], mybir.AluOpType.add
                    )
                    e23 = esp.tile([P, V], BF16, tag="esum")
                    nc.vector.tensor_copy(e23[64:], e_sb[2][64:])
                    nc.vector.tensor_tensor(
                        e23[:wsz3], e_sb[2][:wsz3], e_sb[3][:wsz3],
                        mybir.AluOpType.add,
                    )
                    esum = esp.tile([P, V], BF16, tag="esum")
                    nc.vector.tensor_tensor(
                        esum[:], e01[:], e23[:], mybir.AluOpType.add
                    )
                    z_ps = pp_z.tile([P, V], F32, tag="z")
                    nc.tensor.matmul(
                        z_ps[:], lhsT=ones[:], rhs=esum[:], start=True, stop=True
                    )
                    zinv = zp.tile([P, V], F32, tag="zinv")
                    nc.vector.reciprocal_approx_fast(out=zinv[:], in_=z_ps[:])

                    # U^T = A_unnorm @ g : [OUT(2x128), V]; normalize; stats; spill
                    resident = tb in RESIDENT_TBS
                    o_sb = None if resident else op_.tile([P, 2, V], BF16, tag="o")
                    for oc in range(2):
                        u_ps = pp_u.tile([P, V], F32, tag="u")
                        for wc, (w0, wsz) in enumerate(WCH):
                            nc.tensor.matmul(
                                u_ps[:],
                                lhsT=g_sb[wc][:wsz, oc * P : (oc + 1) * P],
                                rhs=e_sb[wc][:wsz, :],
                                start=(wc == 0), stop=(wc == 3),
                            )
                        o_ap = obig[(oc, tb)][:, tt, :] if resident else o_sb[:, oc, :]
                        nc.vector.tensor_tensor(
                            o_ap, u_ps[:], zinv[:], mybir.AluOpType.mult
                        )
                        nc.vector.bn_stats(
                            out=stats_acc_t[:, t, oc, :], in_=o_ap
                        )
                        if not resident:
                            nc.sync.dma_start(out=scr[oc][tb][:, tt, :], in_=o_sb[:, oc, :])

            # ---- prefetch the first phase-2 input groups (overlaps collective) ----
            tins = {}
            for tb in range(PREFETCH_GROUPS):
                for oc in range(2):
                    tin = p2.tile([P, T_BLK, V], BF16, tag="p2in", bufs=8,
                                  name=f"tin_pre{oc}_{tb}")
                    nc.sync.dma_start(out=tin[:], in_=scr[oc][tb][:])
                    tins[(oc, tb)] = tin

            # ---- BN stats reduce + all-reduce ----
            cnt_local = float(T * V)
            pay = small.tile([P, 4], F32, tag="pay")
            tmp1 = small.tile([P, 2], F32, tag="tmp1")
            tmp2 = small.tile([P, 2], F32, tag="tmp2")
            for oc in range(2):
                mv = small.tile([P, 2], F32, tag=f"mv{oc}", name=f"mv{oc}")
                nc.vector.bn_aggr(out=mv[:], in_=stats_acc_t[:, :, oc, :])
                # sum = cnt*mean ; sumsq = cnt*(var + mean^2)
                nc.vector.tensor_scalar_mul(pay[:, oc : oc + 1], mv[:, 0:1], cnt_local)
                nc.vector.tensor_tensor(
                    tmp1[:, oc : oc + 1], mv[:, 0:1], mv[:, 0:1], mybir.AluOpType.mult
                )
                nc.vector.tensor_tensor(
                    tmp2[:, oc : oc + 1], mv[:, 1:2], tmp1[:, oc : oc + 1],
                    mybir.AluOpType.add,
                )
                nc.vector.tensor_scalar_mul(
                    pay[:, 2 + oc : 3 + oc], tmp2[:, oc : oc + 1], cnt_local
                )

            pay_dram = dram.tile([P, 4], F32)
            red_dram = dram.tile([P, 4], F32)
            nc.gpsimd.dma_start(out=pay_dram[:], in_=pay[:])
            nc.gpsimd.collective_compute(
                "AllReduce",
                mybir.AluOpType.add,
                replica_groups=[list(range(NCORES))],
                ins=[pay_dram.opt()],
                outs=[red_dram.opt()],
            )
            red = small.tile([P, 4], F32, tag="red")
            nc.gpsimd.dma_start(out=red[:], in_=red_dram[:])

            # mean = sum/N ; var = sumsq/N - mean^2 ; s = gamma/sqrt(var+eps)
            # t = beta - mean*s
            mean_g = small.tile([P, 2], F32, tag="mean_g")
            ex2 = small.tile([P, 2], F32, tag="ex2")
            var_g = small.tile([P, 2], F32, tag="var_g")
            rstd = small.tile([P, 2], F32, tag="rstd")
            s_vec = small.tile([P, 2], F32, tag="s_vec")
            t_vec = small.tile([P, 2], F32, tag="t_vec")
            nc.vector.tensor_scalar_mul(mean_g[:], red[:, 0:2], 1.0 / cnt_glob)
            nc.vector.tensor_scalar_mul(ex2[:], red[:, 2:4], 1.0 / cnt_glob)
            nc.vector.tensor_tensor(
                var_g[:], mean_g[:], mean_g[:], mybir.AluOpType.mult
            )
            nc.vector.tensor_tensor(
                var_g[:], ex2[:], var_g[:], mybir.AluOpType.subtract
            )
            nc.scalar.activation(
                rstd[:], var_g[:], mybir.ActivationFunctionType.Sqrt,
                bias=eps_sb[:, 0:1],
            )
            nc.vector.reciprocal(out=rstd[:], in_=rstd[:])
            nc.vector.tensor_tensor(s_vec[:], rstd[:], gb_sb[:, 0:2], mybir.AluOpType.mult)
            nc.vector.tensor_tensor(t_vec[:], mean_g[:], s_vec[:], mybir.AluOpType.mult)
            nc.vector.tensor_tensor(
                t_vec[:], gb_sb[:, 2:4], t_vec[:], mybir.AluOpType.subtract
            )

            # ---- phase 2: scratch -> affine -> out ----
            p2_order = list(RESIDENT_TBS) + [tb for tb in range(N_TB) if tb not in RESIDENT_TBS]
            for tb in p2_order:
                for oc in range(2):
                    t0 = tb * T_BLK
                    if tb in RESIDENT_TBS:
                        tin = obig[(oc, tb)]
                    elif (oc, tb) in tins:
                        tin = tins.pop((oc, tb))
                    else:
                        tin = p2.tile([P, T_BLK, V], BF16, tag="p2in",
                                      bufs=8,
                                      name=f"tin{oc}_{tb}")
                        nc.sync.dma_start(out=tin[:], in_=scr[oc][tb][:])
                    half = T_BLK // 2
                    for h in range(2):
                        tout = p2.tile([P, half, V], F32, tag="p2out", bufs=3,
                                       name=f"tout{oc}_{tb}_{h}")
                        nc.vector.tensor_scalar(
                            tout[:], tin[:, h * half : (h + 1) * half, :],
                            s_vec[:, oc : oc + 1], t_vec[:, oc : oc + 1],
                            mybir.AluOpType.mult, mybir.AluOpType.add,
                        )
                        nc.sync.dma_start(
                            out=out_ext[oc * P : (oc + 1) * P,
                                        t0 + h * half : t0 + (h + 1) * half, :],
                            in_=tout[:],
                        )

    nc.compile()
    return nc


def _get_nc(with_bias: bool):
    key = with_bias
    if key not in _CACHE:
        _CACHE[key] = _build(with_bias)
    return _CACHE[key]


def _ensure_ntff_hook():
    import sys, types
    import antenv

    if "antenv.axon_hooks" not in sys.modules:
        mod = types.ModuleType("antenv.axon_hooks")
        _h = [None]
        mod.set_axon_ntff_profile_hook = lambda h: _h.__setitem__(0, h)
        mod.get_axon_ntff_profile_hook = lambda: _h[0]
        sys.modules["antenv.axon_hooks"] = mod
        antenv.axon_hooks = mod
    mod = sys.modules["antenv.axon_hooks"]
    if mod.get_axon_ntff_profile_hook() is None:
        try:
            from trn_agent_boot.trn_boot import _ntff_profile_via_ctypes

            mod.set_axon_ntff_profile_hook(
                _ntff_profile_via_ctypes("/opt/axon/libaxon_pjrt.so")
            )
        except Exception:
            pass


def kernel(x, W_theta, b_theta, W_phi, b_phi, W_g, b_g, bn_gamma, bn_beta):
    global LAST_EXEC_NS
    x = np.asarray(x, dtype=np.float32)
    with_bias = bool(
        np.any(np.asarray(b_theta)) or np.any(np.asarray(b_phi)) or np.any(np.asarray(b_g))
    )

    x_bf = x.astype(ml_dtypes.bfloat16)  # (B, C, T, V)
    wt = np.ascontiguousarray(
        np.asarray(W_theta, dtype=np.float32).T.astype(ml_dtypes.bfloat16).reshape(2, P, INTER)
    )
    wp = np.ascontiguousarray(
        np.asarray(W_phi, dtype=np.float32).T.astype(ml_dtypes.bfloat16).reshape(2, P, INTER)
    )
    wg = np.ascontiguousarray(
        np.asarray(W_g, dtype=np.float32).T.astype(ml_dtypes.bfloat16).reshape(2, P, OUT)
    )
    gamma = np.asarray(bn_gamma, dtype=np.float32).reshape(2, P).T  # [128, 2]
    beta = np.asarray(bn_beta, dtype=np.float32).reshape(2, P).T
    gb = np.ascontiguousarray(np.concatenate([gamma, beta], axis=1))  # [128, 4]

    nc = _get_nc(with_bias)

    in_maps = []
    for b in range(NCORES):
        m = {
            "x": np.ascontiguousarray(x_bf[b]),
            "wt": wt,
            "wp": wp,
            "wg": wg,
            "gb": gb,
        }
        if with_bias:
            m["bt"] = np.asarray(b_theta, dtype=np.float32).reshape(INTER, 1)
            m["bp"] = np.asarray(b_phi, dtype=np.float32).reshape(INTER, 1)
            m["bg"] = np.asarray(b_g, dtype=np.float32).reshape(1, OUT)
        in_maps.append(m)

    if TRACE:
        _ensure_ntff_hook()
    r = run_bass_kernel_spmd(nc, in_maps, list(range(NCORES)), trace=TRACE)
    LAST_EXEC_NS = r.exec_time_ns

    out = np.stack([r.results[b]["out"] for b in range(NCORES)], axis=0)
    return out.astype(np.float32)


# revision 15
# speedup vs baseline: 1.0506x; 1.0015x over previous
"""Trainium2 Bass kernel for AdaptiveGraphConv (per-(b,t) graph attention + BatchNorm2d).

Reference math (B=8, C=256, T=64, V=468, INTER=128, OUT=256):
    theta = einsum('bctv,ic->btvi', x, W_theta) + b_theta
    phi   = einsum('bctv,ic->btvi', x, W_phi)   + b_phi
    g     = einsum('bctv,oc->btvo', x, W_g)     + b_g
    A     = softmax(theta @ phi^T / sqrt(INTER), axis=-1)   # per (b,t), V x V
    out   = (A @ g) transposed to (B, OUT, T, V)
    out   = batchnorm2d(out, training stats over (B,T,V) per channel)

Sharding: data-parallel over B (1 batch per NeuronCore, 8 cores);
BN batch statistics are all-reduced across cores.

Device program (SPMD, per core, matmuls bf16 with fp32 PSUM accumulation):
  phase 1 (per t slice):
    thetaT[i,v], phiT[i,v] = W^T-chunks x x-chunks           (PE)
    S^T[w,v] = phiT-cols^T @ thetaT ; E^T = exp(S^T/sqrt(d)) (PE + ACT)
    g[w,o] = x-cols^T @ WgT                                  (PE + ACT/DVE cast)
    esum = sum of E^T chunks                                 (DVE)
    Z = partition_all_reduce(esum)  (broadcast to 128 parts) (GpSimd)
    zinv = reciprocal_approx(Z)                              (DVE)
    U^T[o,v] = g-cols^T @ E^T                                (PE)
    out = U^T * zinv  (+ channel sum via fused reduce)       (DVE TTR)
    sumsq via fused square-reduce                            (DVE TTR)
    out -> DRAM scratch (bf16)
  all-reduce (sum, sumsq) per channel -> BN affine (s, t)
  phase 2: stream scratch, out*s+t per channel, write output (DVE + DMA)
"""

import math

import numpy as np
import ml_dtypes

import concourse.bacc as bacc
import concourse.tile as tile
from concourse import mybir
from concourse.bass_utils import run_bass_kernel_spmd

B, C, T, V = 8, 256, 64, 468
INTER, OUT = 128, 256
BN_EPS = 1e-5
NCORES = 8
P = 128

SCALE = 1.0 / math.sqrt(INTER)
# w-axis chunks of V for 128-partition tiles
WCH = [(0, 128), (128, 128), (256, 128), (384, V - 384)]
T_BLK = 8  # t-slices per input DMA / phase-2 tile
N_TB = T // T_BLK
PREFETCH_GROUPS = 3
RESIDENT_TBS = [4, 5, 6, 7]  # t-groups whose outputs stay in SBUF (no scratch round-trip)  # (tb) groups x 2 oc tiles prefetched before the collective

F32 = mybir.dt.float32
BF16 = mybir.dt.bfloat16

TRACE = False
LAST_EXEC_NS = None

_CACHE = {}


def _build(with_bias: bool):
    nc = bacc.Bacc("TRN2", target_bir_lowering=False, debug=False, num_devices=NCORES)

    x_ext = nc.dram_tensor("x", [C, T, V], BF16, kind="ExternalInput").ap()
    wt_ext = nc.dram_tensor("wt", [2, P, INTER], BF16, kind="ExternalInput").ap()
    wp_ext = nc.dram_tensor("wp", [2, P, INTER], BF16, kind="ExternalInput").ap()
    wg_ext = nc.dram_tensor("wg", [2, P, OUT], BF16, kind="ExternalInput").ap()
    gb_ext = nc.dram_tensor("gb", [P, 4], F32, kind="ExternalInput").ap()
    if with_bias:
        bt_ext = nc.dram_tensor("bt", [INTER, 1], F32, kind="ExternalInput").ap()
        bp_ext = nc.dram_tensor("bp", [INTER, 1], F32, kind="ExternalInput").ap()
        bg_ext = nc.dram_tensor("bg", [1, OUT], F32, kind="ExternalInput").ap()
    out_ext = nc.dram_tensor("out", [OUT, T, V], F32, kind="ExternalOutput").ap()

    cnt_glob = float(NCORES * T * V)

    with tile.TileContext(nc) as tc:
        with (
            tc.tile_pool(name="consts", bufs=1) as consts,
            tc.tile_pool(name="xin", bufs=2) as xin,
            tc.tile_pool(name="thp", bufs=4) as thp,
            tc.tile_pool(name="ep", bufs=8) as ep,
            tc.tile_pool(name="gp", bufs=8) as gp,
            tc.tile_pool(name="esp", bufs=6) as esp,
            tc.tile_pool(name="zp", bufs=2) as zp,
            tc.tile_pool(name="op", bufs=4) as op_,
            tc.tile_pool(name="small", bufs=1) as small,
            tc.tile_pool(name="p2", bufs=2) as p2,
            tc.tile_pool(name="pp_th", bufs=2, space="PSUM") as pp_th,
            tc.tile_pool(name="pp_sg", bufs=3, space="PSUM") as pp_sg,
            tc.tile_pool(name="pp_z", bufs=1, space="PSUM") as pp_z,
            tc.tile_pool(name="pp_u", bufs=2, space="PSUM") as pp_u,
            tc.tile_pool(name="dram", bufs=1, space="DRAM") as dram,
        ):
            # ---- constants ----
            wt_sb = [consts.tile([P, INTER], BF16, tag=f"wt{k}", name=f"wt_sb{k}") for k in range(2)]
            wp_sb = [consts.tile([P, INTER], BF16, tag=f"wp{k}", name=f"wp_sb{k}") for k in range(2)]
            wg_sb = [consts.tile([P, OUT], BF16, tag=f"wg{k}", name=f"wg_sb{k}") for k in range(2)]
            for k in range(2):
                nc.sync.dma_start(out=wt_sb[k][:], in_=wt_ext[k])
                nc.sync.dma_start(out=wp_sb[k][:], in_=wp_ext[k])
                nc.sync.dma_start(out=wg_sb[k][:], in_=wg_ext[k])
            ones = consts.tile([P, P], BF16, tag="ones")
            nc.vector.memset(ones[:], 1.0)
            gb_sb = consts.tile([P, 4], F32, tag="gb")
            nc.sync.dma_start(out=gb_sb[:], in_=gb_ext[:])
            eps_sb = consts.tile([P, 1], F32, tag="eps")
            nc.vector.memset(eps_sb[:], BN_EPS)
            warm = consts.tile([P, 1], F32, tag="warm")
            nc.scalar.activation(warm[:], eps_sb[:], mybir.ActivationFunctionType.Exp)
            if with_bias:
                bt_sb = consts.tile([INTER, 1], F32, tag="bt")
                bp_sb = consts.tile([INTER, 1], F32, tag="bp")
                bg_sb = consts.tile([P, OUT], F32, tag="bg")
                nc.sync.dma_start(out=bt_sb[:], in_=bt_ext[:])
                nc.sync.dma_start(out=bp_sb[:], in_=bp_ext[:])
                nc.sync.dma_start(out=bg_sb[:], in_=bg_ext.to_broadcast([P, OUT]))

            # per-channel running stats (bn_stats 6-tuples per t-slice and o-chunk)
            stats_acc_t = consts.tile([P, T, 2, 6], F32, tag="stats", name="stats_acc_t")

            scr = [
                [dram.tile([P, T_BLK, V], BF16, name=f"scr{oc}_{tb}") for tb in range(N_TB)]
                for oc in range(2)
            ]
            obig = {
                (oc, tb): consts.tile([P, T_BLK, V], BF16, tag=f"obig{oc}_{tb}",
                                      name=f"obig{oc}_{tb}")
                for oc in range(2) for tb in RESIDENT_TBS
            }

            # ---- phase 1 ----
            for tb in range(N_TB):
                xg = [xin.tile([P, T_BLK, V], BF16, tag=f"xg{k}", name=f"xg{k}") for k in range(2)]
                for k in range(2):
                    nc.sync.dma_start(
                        out=xg[k][:],
                        in_=x_ext[k * P : (k + 1) * P, tb * T_BLK : (tb + 1) * T_BLK, :],
                    )
                for tt in range(T_BLK):
                    t = tb * T_BLK + tt
                    xt = [xg[k][:, tt, :] for k in range(2)]

                    # thetaT / phiT : [INTER, V]
                    th_ps = pp_th.tile([P, V], F32, tag="th")
                    ph_ps = pp_th.tile([P, V], F32, tag="th")
                    for k in range(2):
                        nc.tensor.matmul(
                            th_ps[:], lhsT=wt_sb[k][:], rhs=xt[k],
                            start=(k == 0), stop=(k == 1),
                        )
                    for k in range(2):
                        nc.tensor.matmul(
                            ph_ps[:], lhsT=wp_sb[k][:], rhs=xt[k],
                            start=(k == 0), stop=(k == 1),
                        )
                    th_sb = thp.tile([P, V], BF16, tag="th_sb")
                    ph_sb = thp.tile([P, V], BF16, tag="th_sb")
                    if with_bias:
                        nc.scalar.activation(
                            th_sb[:], th_ps[:], mybir.ActivationFunctionType.Identity,
                            bias=bt_sb[:, 0:1],
                        )
                        nc.scalar.activation(
                            ph_sb[:], ph_ps[:], mybir.ActivationFunctionType.Identity,
                            bias=bp_sb[:, 0:1],
                        )
                    else:
                        nc.scalar.copy(th_sb[:], th_ps[:])
                        nc.scalar.copy(ph_sb[:], ph_ps[:])

                    # scores^T chunks + exp
                    e_sb = []
                    for (w0, wsz) in WCH:
                        s_ps = pp_sg.tile([P, V], F32, tag="sg")
                        nc.tensor.matmul(
                            s_ps[:wsz], lhsT=ph_sb[:, w0 : w0 + wsz], rhs=th_sb[:],
                            start=True, stop=True,
                        )
                        e_t = ep.tile([P, V], BF16, tag="e")
                        nc.scalar.activation(
                            e_t[:wsz], s_ps[:wsz],
                            mybir.ActivationFunctionType.Exp, scale=SCALE,
                        )
                        e_sb.append(e_t)

                    # g chunks: [w, OUT]
                    g_sb = []
                    for (w0, wsz) in WCH:
                        g_ps = pp_sg.tile([P, OUT], F32, tag="sg")
                        for k in range(2):
                            nc.tensor.matmul(
                                g_ps[:wsz], lhsT=xt[k][:, w0 : w0 + wsz], rhs=wg_sb[k][:],
                                start=(k == 0), stop=(k == 1),
                            )
                        g_t = gp.tile([P, OUT], BF16, tag="g")
                        if with_bias:
                            nc.vector.tensor_tensor(
                                g_t[:wsz], g_ps[:wsz], bg_sb[:wsz],
                                mybir.AluOpType.add,
                            )
                        else:
                            nc.scalar.copy(g_t[:wsz], g_ps[:wsz])
                        g_sb.append(g_t)

                    # softmax denominator: pre-fold E chunks pairwise on DVE,
                    # then 2 ones-matmuls reduce over partitions and broadcast
                    wsz3 = WCH[3][1]
                    e01 = esp.tile([P, V], BF16, tag="esum")
                    nc.vector.tensor_tensor(
                        e01[:], e_sb[0][:], e_sb[1][:], mybir.AluOpType.add
                    )
                    e23 = esp.tile([P, V], BF16, tag="esum")
                    nc.vector.tensor_copy(e23[64:], e_sb[2][64:])
                    nc.vector.tensor_tensor(
                        e23[:wsz3], e_sb[2][:wsz3], e_sb[3][:wsz3],
                        mybir.AluOpType.add,
                    )
                    esum = esp.tile([P, V], BF16, tag="esum")
                    nc.vector.tensor_tensor(
                        esum[:], e01[:], e23[:], mybir.AluOpType.add
                    )
                    z_ps = pp_z.tile([P, V], F32, tag="z")
                    nc.tensor.matmul(
                        z_ps[:], lhsT=ones[:], rhs=esum[:], start=True, stop=True
                    )
                    zinv = zp.tile([P, V], F32, tag="zinv")
                    nc.vector.reciprocal_approx_fast(out=zinv[:], in_=z_ps[:])

                    # U^T = A_unnorm @ g : [OUT(2x128), V]; normalize; stats; spill
                    resident = tb in RESIDENT_TBS
                    o_sb = None if resident else op_.tile([P, 2, V], BF16, tag="o")
                    for oc in range(2):
                        u_ps = pp_u.tile([P, V], F32, tag="u")
                        for wc, (w0, wsz) in enumerate(WCH):
                            nc.tensor.matmul(
                                u_ps[:],
                                lhsT=g_sb[wc][:wsz, oc * P : (oc + 1) * P],
                                rhs=e_sb[wc][:wsz, :],
                                start=(wc == 0), stop=(wc == 3),
                            )
                        o_ap = obig[(oc, tb)][:, tt, :] if resident else o_sb[:, oc, :]
                        nc.vector.tensor_tensor(
                            o_ap, u_ps[:], zinv[:], mybir.AluOpType.mult
                        )
                        nc.vector.bn_stats(
                            out=stats_acc_t[:, t, oc, :], in_=o_ap
                        )
                        if not resident:
                            nc.sync.dma_start(out=scr[oc][tb][:, tt, :], in_=o_sb[:, oc, :])

            # ---- prefetch the first phase-2 input groups (overlaps collective) ----
            tins = {}
            for tb in range(PREFETCH_GROUPS):
                for oc in range(2):
                    tin = p2.tile([P, T_BLK, V], BF16, tag="p2in", bufs=7,
                                  name=f"tin_pre{oc}_{tb}")
                    nc.sync.dma_start(out=tin[:], in_=scr[oc][tb][:])
                    tins[(oc, tb)] = tin

            # ---- BN stats reduce + all-reduce ----
            cnt_local = float(T * V)
            pay = small.tile([P, 4], F32, tag="pay")
            tmp1 = small.tile([P, 2], F32, tag="tmp1")
            tmp2 = small.tile([P, 2], F32, tag="tmp2")
            for oc in range(2):
                mv = small.tile([P, 2], F32, tag=f"mv{oc}", name=f"mv{oc}")
                nc.vector.bn_aggr(out=mv[:], in_=stats_acc_t[:, :, oc, :])
                # sum = cnt*mean ; sumsq = cnt*(var + mean^2)
                nc.vector.tensor_scalar_mul(pay[:, oc : oc + 1], mv[:, 0:1], cnt_local)
                nc.vector.tensor_tensor(
                    tmp1[:, oc : oc + 1], mv[:, 0:1], mv[:, 0:1], mybir.AluOpType.mult
                )
                nc.vector.tensor_tensor(
                    tmp2[:, oc : oc + 1], mv[:, 1:2], tmp1[:, oc : oc + 1],
                    mybir.AluOpType.add,
                )
                nc.vector.tensor_scalar_mul(
                    pay[:, 2 + oc : 3 + oc], tmp2[:, oc : oc + 1], cnt_local
                )

            pay_dram = dram.tile([P, 4], F32)
            red_dram = dram.tile([P, 4], F32)
            nc.gpsimd.dma_start(out=pay_dram[:], in_=pay[:])
            nc.gpsimd.collective_compute(
                "AllReduce",
                mybir.AluOpType.add,
                replica_groups=[list(range(NCORES))],
                ins=[pay_dram.opt()],
                outs=[red_dram.opt()],
            )
            red = small.tile([P, 4], F32, tag="red")
            nc.gpsimd.dma_start(out=red[:], in_=red_dram[:])

            # mean = sum/N ; var = sumsq/N - mean^2 ; s = gamma/sqrt(var+eps)
            # t = beta - mean*s
            mean_g = small.tile([P, 2], F32, tag="mean_g")
            ex2 = small.tile([P, 2], F32, tag="ex2")
            var_g = small.tile([P, 2], F32, tag="var_g")
            rstd = small.tile([P, 2], F32, tag="rstd")
            s_vec = small.tile([P, 2], F32, tag="s_vec")
            t_vec = small.tile([P, 2], F32, tag="t_vec")
            nc.vector.tensor_scalar_mul(mean_g[:], red[:, 0:2], 1.0 / cnt_glob)
            nc.vector.tensor_scalar_mul(ex2[:], red[:, 2:4], 1.0 / cnt_glob)
            nc.vector.tensor_tensor(
                var_g[:], mean_g[:], mean_g[:], mybir.AluOpType.mult
            )
            nc.vector.tensor_tensor(
                var_g[:], ex2[:], var_g[:], mybir.AluOpType.subtract
            )
            nc.scalar.activation(
                rstd[:], var_g[:], mybir.ActivationFunctionType.Sqrt,
                bias=eps_sb[:, 0:1],
            )
            nc.vector.reciprocal(out=rstd[:], in_=rstd[:])
            nc.vector.tensor_tensor(s_vec[:], rstd[:], gb_sb[:, 0:2], mybir.AluOpType.mult)
            nc.vector.tensor_tensor(t_vec[:], mean_g[:], s_vec[:], mybir.AluOpType.mult)
            nc.vector.tensor_tensor(
                t_vec[:], gb_sb[:, 2:4], t_vec[:], mybir.AluOpType.subtract
            )

            # ---- phase 2: scratch -> affine -> out ----
            p2_order = list(RESIDENT_TBS) + [tb for tb in range(N_TB) if tb not in RESIDENT_TBS]
            for tb in p2_order:
                for oc in range(2):
                    t0 = tb * T_BLK
                    if tb in RESIDENT_TBS:
                        tin = obig[(oc, tb)]
                    elif (oc, tb) in tins:
                        tin = tins.pop((oc, tb))
                    else:
                        tin = p2.tile([P, T_BLK, V], BF16, tag="p2in",
                                      bufs=7,
                                      name=f"tin{oc}_{tb}")
                        nc.sync.dma_start(out=tin[:], in_=scr[oc][tb][:])
                    half = T_BLK // 2
                    for h in range(2):
                        tout = p2.tile([P, half, V], F32, tag="p2out", bufs=3,
                                       name=f"tout{oc}_{tb}_{h}")
                        nc.vector.tensor_scalar(
                            tout[:], tin[:, h * half : (h + 1) * half, :],
                            s_vec[:, oc : oc + 1], t_vec[:, oc : oc + 1],
                            mybir.AluOpType.mult, mybir.AluOpType.add,
                        )
                        nc.sync.dma_start(
                            out=out_ext[oc * P : (oc + 1) * P,
                                        t0 + h * half : t0 + (h + 1) * half, :],
                            in_=tout[:],
                        )

    nc.compile()
    return nc


def _get_nc(with_bias: bool):
    key = with_bias
    if key not in _CACHE:
        _CACHE[key] = _build(with_bias)
    return _CACHE[key]


def _ensure_ntff_hook():
    import sys, types
    import antenv

    if "antenv.axon_hooks" not in sys.modules:
        mod = types.ModuleType("antenv.axon_hooks")
        _h = [None]
        mod.set_axon_ntff_profile_hook = lambda h: _h.__setitem__(0, h)
        mod.get_axon_ntff_profile_hook = lambda: _h[0]
        sys.modules["antenv.axon_hooks"] = mod
        antenv.axon_hooks = mod
    mod = sys.modules["antenv.axon_hooks"]
    if mod.get_axon_ntff_profile_hook() is None:
        try:
            from trn_agent_boot.trn_boot import _ntff_profile_via_ctypes

            mod.set_axon_ntff_profile_hook(
                _ntff_profile_via_ctypes("/opt/axon/libaxon_pjrt.so")
            )
        except Exception:
            pass


def kernel(x, W_theta, b_theta, W_phi, b_phi, W_g, b_g, bn_gamma, bn_beta):
    global LAST_EXEC_NS
    x = np.asarray(x, dtype=np.float32)
    with_bias = bool(
        np.any(np.asarray(b_theta)) or np.any(np.asarray(b_phi)) or np.any(np.asarray(b_g))
    )

    x_bf = x.astype(ml_dtypes.bfloat16)  # (B, C, T, V)
    wt = np.ascontiguousarray(
        np.asarray(W_theta, dtype=np.float32).T.astype(ml_dtypes.bfloat16).reshape(2, P, INTER)
    )
    wp = np.ascontiguousarray(
        np.asarray(W_phi, dtype=np.float32).T.astype(ml_dtypes.bfloat16).reshape(2, P, INTER)
    )
    wg = np.ascontiguousarray(
        np.asarray(W_g, dtype=np.float32).T.astype(ml_dtypes.bfloat16).reshape(2, P, OUT)
    )
    gamma = np.asarray(bn_gamma, dtype=np.float32).reshape(2, P).T  # [128, 2]
    beta = np.asarray(bn_beta, dtype=np.float32).reshape(2, P).T
    gb = np.ascontiguousarray(np.concatenate([gamma, beta], axis=1))  # [128, 4]

    nc = _get_nc(with_bias)

    in_maps = []
    for b in range(NCORES):
        m = {
            "x": np.ascontiguousarray(x_bf[b]),
            "wt": wt,
            "wp": wp,
            "wg": wg,
            "gb": gb,
        }
        if with_bias:
            m["bt"] = np.asarray(b_theta, dtype=np.float32).reshape(INTER, 1)
            m["bp"] = np.asarray(b_phi, dtype=np.float32).reshape(INTER, 1)
            m["bg"] = np.asarray(b_g, dtype=np.float32).reshape(1, OUT)
        in_maps.append(m)

    if TRACE:
        _ensure_ntff_hook()
    r = run_bass_kernel_spmd(nc, in_maps, list(range(NCORES)), trace=TRACE)
    LAST_EXEC_NS = r.exec_time_ns

    out = np.stack([r.results[b]["out"] for b in range(NCORES)], axis=0)
    return out.astype(np.float32)
